# revision 38
# baseline (speedup 1.0000x reference)
"""Trainium2 Bass kernel for the pointer-generator decoder step.

Contract: kernel(**inputs) takes the FULL unsharded inputs (as produced by the
problem's setup_inputs()) and returns the FULL [B, V+OOV] output.

Sharding (8 NeuronCores, one SPMD launch):
  * Front end (LSTM step, attention, context, p_gen, fc1) is data-parallel
    over batch (32 rows/core).
  * z1 = fc1 output is AllGathered (bf16, 0.5 MB) so every core holds the
    full batch.
  * fc2 is tensor-parallel over vocab: each core computes exp(logits) for its
    6250-column slice of the full batch plus partial softmax denominators.
    Denominator combination, gen-scaling, OOV extension and the copy
    scatter-add run on the host during output assembly.

Performance structure (vs the first working version):
  * The two big streams (encoder_outputs ~6.5MB, fc2 weights ~6.4MB per core)
    are host-prepacked into exact SBUF-image layouts so each DMA descriptor
    covers a full partition line (4-13KB) instead of 800B -- the DMA queues
    run at bandwidth instead of descriptor rate.
  * enco stream + attention/context run first; the fc2 weight stream and the
    z1 AllGather overlap with them on separate DGE rings.
  * fc2 weights and z1 are fp8e4 (scaled) with DoubleRow matmuls (2 MACs/
    cell/cycle); enco and the attention vector are fp8e4 too.  The attention
    softmax path (which dominates the output's absmax via att_copy) stays
    fp32/f32r, so the extra fp8 error only touches the small p_vocab entries.
"""

import os
import sys

for _p in ("/opt/trn_rl_repo",):
    if _p not in sys.path and os.path.isdir(_p):
        sys.path.insert(0, _p)

import ml_dtypes
import numpy as np

import concourse.bass as bass
import concourse.bacc as bacc_mod
import concourse.mybir as mybir
import concourse.tile as tile
from concourse.bass_utils import run_bass_kernel_spmd
from concourse.masks import make_identity

NCORES = 8
B = 256           # batch
BC = B // NCORES  # batch shard per core (32)
I = 256           # input dim
H = 512           # hidden dim
A = 400           # attention dim
V = 50000         # vocab
VC = V // NCORES  # vocab shard per core (6250)
NT = 512          # vocab tile (psum bank) size
NVT = 13          # vocab tiles per core (13*512 = 6656, 406 padded cols)
VCP = NVT * NT    # padded vocab shard

# precision knobs
FC2_FP8 = True    # fc2 weights + z1 in fp8e4, DoubleRow matmuls
ENCO_FP8 = True   # encoder_outputs + att in fp8e4, DoubleRow matmuls

# fp8 scales (values are pushed into e4m3's normal range; descaled on-chip)
S_Z1 = 16.0
S_W2 = 64.0
S_ATT = 32.0
S_ENC = 16.0

F32 = mybir.dt.float32
F32R = mybir.dt.float32r
BF16 = mybir.dt.bfloat16
FP8 = mybir.dt.float8e4
AF = mybir.ActivationFunctionType
ALU = mybir.AluOpType
AX = mybir.AxisListType
DR = mybir.MatmulPerfMode.DoubleRow

# 400 split into PE-friendly chunks (for fc1's ctx rows)
CH4 = [(0, 128), (128, 128), (256, 128), (384, 16)]

# packed small-weight image column offsets
SW_WIH = 0            # 2*1536
SW_X0T = 3072         # 2*32
SW_BG = 3136          # 12
SW_EST = 3148         # 4*32
SW_WHSW = 3276        # 4*800
NW = 6476


def _bc(ap, parts):
    """Broadcast a DRAM AP across `parts` partitions (0-stride partition dim)."""
    return bass.AP(tensor=ap.tensor, offset=ap.offset, ap=[[0, parts]] + list(ap.ap))


def build_nc(with_fc1_bias: bool) -> bass.Bass:
    nc = bacc_mod.Bacc("TRN2", target_bir_lowering=False, num_devices=NCORES)

    enc_dt = FP8 if ENCO_FP8 else BF16
    w2_dt = FP8 if FC2_FP8 else BF16

    # ---- external inputs ----
    x0 = nc.dram_tensor("x0", [BC, I], F32, kind="ExternalInput")
    # packed per-partition image of all small weights (one DMA, one
    # completion sem): wih(2*1536) | x0T(2*32) | bg(12) | esT(4*32) |
    # whsw(4*800); see SW_* offsets
    smallw = nc.dram_tensor("smallw", [128, NW], BF16, kind="ExternalInput")
    # enco image: [group of 4 batches][partition=a%128][b4*4 + achunk][e]
    enco = nc.dram_tensor("enco", [8, 128, 16, A], enc_dt, kind="ExternalInput")
    # packed: attb(400) | vvec(400) | pg2(400) | pg1(256) | pg3(512)
    smallp = nc.dram_tensor("smallp", [1968], F32, kind="ExternalInput")
    # fc1 weight image: [partition][chunk 0-3 ctx, 4-7 h][n]
    fc1ab = nc.dram_tensor("fc1ab", [128, 8, 2 * H], BF16, kind="ExternalInput")
    # fc2 weight image: [vocab tile][partition=k%128][kchunk][j]
    fc2w = nc.dram_tensor("fc2w", [NVT, 128, 8, NT], w2_dt, kind="ExternalInput")
    if with_fc1_bias:
        fc1bias = nc.dram_tensor("fc1bias", [2 * H], F32, kind="ExternalInput")

    # ---- external outputs ----
    # p image: [batch half][partition=b%128][vocab tile][j]
    p_img = nc.dram_tensor("p_img", [2, 128, NVT, NT], FP8 if FC2_FP8 else BF16, kind="ExternalOutput")
    attcopy_out = nc.dram_tensor("attcopy_out", [BC, A], F32, kind="ExternalOutput")
    s_out = nc.dram_tensor("s_out", [B, 1], F32, kind="ExternalOutput")
    gen_out = nc.dram_tensor("gen_out", [BC, 1], F32, kind="ExternalOutput")

    RG = [list(range(NCORES))]

    from contextlib import ExitStack

    with tile.TileContext(nc) as tc, ExitStack() as ctx:
        dram = ctx.enter_context(tc.tile_pool(name="dram", bufs=1, space="DRAM"))
        ag_dt = FP8 if FC2_FP8 else BF16
        z1g_c = dram.tile([BC, 1024], ag_dt)
        z1g_full = dram.tile([B, 1024], ag_dt, addr_space="Shared")

        const = ctx.enter_context(tc.tile_pool(name="const", bufs=1))
        small = ctx.enter_context(tc.tile_pool(name="small", bufs=4))
        psA = ctx.enter_context(tc.tile_pool(name="psA", bufs=3, space="PSUM"))
        psT = ctx.enter_context(tc.tile_pool(name="psT", bufs=1, space="PSUM"))

        ident = const.tile([128, 128], F32)
        make_identity(nc, ident)
        ident_bf = const.tile([128, 128], BF16)
        make_identity(nc, ident_bf)

        # pre-warm the scalar engine's sigmoid/tanh table during the DMA wait
        warm_sb = const.tile([1, 2], F32)
        nc.vector.memset(warm_sb, 0.0)
        nc.scalar.activation(out=warm_sb, in_=warm_sb, func=AF.Sigmoid)

        # ---- constant loads ----
        # sync ring: LSTM/attention weights, then enco stream, then fc2w stream
        sw_sb = const.tile([128, NW], BF16)
        nc.sync.dma_start(out=sw_sb, in_=smallw[:])
        # gpsimd ring: broadcast/small loads (independent of everything)
        x0_sb = const.tile([BC, I], F32)
        nc.gpsimd.dma_start(out=x0_sb, in_=x0[:])
        smallc = const.tile([BC, 1968], F32)
        nc.gpsimd.dma_start(out=smallc, in_=_bc(smallp[:], BC))
        attb_sb = smallc[:, 0:400]
        v_sb = smallc[:, 400:800]
        pg2_sb = smallc[:, 800:1200]
        pg1_sb = smallc[:, 1200:1456]
        pg3_sb = smallc[:, 1456:1968]
        if with_fc1_bias:
            fc1bias_sb = const.tile([BC, 2 * H], F32)
            nc.gpsimd.dma_start(out=fc1bias_sb, in_=_bc(fc1bias[:], BC))

        # ---- LSTM step (h only; c0=h0=0 so the f-gate and W_hh are dead) ----
        sg_sb = const.tile([128, 12, BC], F32)  # sig(i), tanh(g), sig(o)
        for m in range(12):
            ps_g = psA.tile([128, BC], F32, tag="mmA")
            for k in range(2):
                nc.tensor.matmul(
                    out=ps_g,
                    lhsT=sw_sb[:, SW_WIH + k * 1536 + m * 128:SW_WIH + k * 1536 + (m + 1) * 128],
                    rhs=sw_sb[:, SW_X0T + k * BC:SW_X0T + (k + 1) * BC],
                    start=(k == 0),
                    stop=(k == 1),
                )
            func = AF.Tanh if 4 <= m < 8 else AF.Sigmoid
            nc.scalar.activation(
                out=sg_sb[:, m, :], in_=ps_g, func=func,
                bias=sw_sb[:, SW_BG + m:SW_BG + m + 1], scale=1.0,
            )
        cth_sb = const.tile([128, 4, BC], F32)  # tanh(c)
        nc.vector.tensor_mul(out=cth_sb, in0=sg_sb[:, 0:4, :], in1=sg_sb[:, 4:8, :])
        nc.scalar.activation(out=cth_sb, in_=cth_sb, func=AF.Tanh)
        hT_sb = const.tile([128, 4, BC], F32)  # h feature-major
        nc.vector.tensor_mul(out=hT_sb, in0=sg_sb[:, 8:12, :], in1=cth_sb)
        hT_bf = const.tile([128, 4, BC], BF16)  # h feature-major (attn/fc1 lhsT)
        nc.scalar.copy(out=hT_bf, in_=hT_sb)

        # h batch-major [32, 512] (for the p_gen dot)
        h_sb = const.tile([BC, H], F32)
        for k in range(4):
            ps_t = psT.tile([BC, 128], F32, tag="tp")
            nc.tensor.transpose(ps_t, hT_sb[:, k, :], ident)
            nc.scalar.copy(out=h_sb[:, k * 128:(k + 1) * 128], in_=ps_t)

        # ---- attention scores e = tanh(es @ WhwT + h @ WswT + attb)  [32,400]
        ps_e = psA.tile([BC, A], F32, tag="mmA")
        for k in range(4):
            nc.tensor.matmul(
                out=ps_e,
                lhsT=sw_sb[:, SW_EST + k * BC:SW_EST + (k + 1) * BC],
                rhs=sw_sb[:, SW_WHSW + k * 800:SW_WHSW + k * 800 + A],
                start=(k == 0), stop=False,
            )
        for k in range(4):
            nc.tensor.matmul(
                out=ps_e, lhsT=hT_bf[:, k, :],
                rhs=sw_sb[:, SW_WHSW + k * 800 + A:SW_WHSW + (k + 1) * 800],
                start=False, stop=(k == 3),
            )
        e_sb = const.tile([BC, A], F32)
        nc.vector.scalar_tensor_tensor(
            out=e_sb, in0=ps_e, scalar=1.0, in1=attb_sb,
            op0=ALU.mult, op1=ALU.add,
        )
        nc.scalar.activation(out=e_sb, in_=e_sb, func=AF.Tanh)
        # tanh(e) is in [-1,1], so softmax needs no max-subtraction
        ssum = small.tile([BC, 1], F32)
        nc.scalar.activation(
            out=e_sb, in_=e_sb, func=AF.Exp, accum_out=ssum,
        )
        rs = small.tile([BC, 1], F32)
        nc.vector.reciprocal(out=rs, in_=ssum)
        att_sb = const.tile([BC, 512], F32)
        nc.vector.memset(att_sb[:, A:512], 0.0)
        nc.vector.scalar_tensor_tensor(
            out=att_sb[:, 0:A], in0=e_sb, scalar=rs, in1=v_sb,
            op0=ALU.mult, op1=ALU.mult,
        )

        # fc1 weight loads (scalar ring, issued here so their bandwidth lands
        # after the LSTM/attention weights but well before fc1 needs them)
        fc1ab_sb = const.tile([128, 8, 2 * H], BF16)
        nc.scalar.dma_start(out=fc1ab_sb, in_=fc1ab[:])
        fc1a_sb = fc1ab_sb[:, 0:4, :]
        fc1b_sb = fc1ab_sb[:, 4:8, :]

        # att feature-major for the context matmuls, 4x128 over padded 512
        att_dt = FP8 if ENCO_FP8 else BF16
        attT_sb = const.tile([128, 4, BC], att_dt)
        for t in range(4):
            ps_t = psT.tile([128, BC], F32, tag="tp")
            nc.tensor.transpose(
                ps_t, att_sb[:, t * 128:(t + 1) * 128], ident[:BC, :BC]
            )
            if ENCO_FP8:
                nc.scalar.activation(
                    out=attT_sb[:, t, :], in_=ps_t, func=AF.Identity, scale=S_ATT
                )
            else:
                nc.scalar.copy(out=attT_sb[:, t, :], in_=ps_t)

        # ---- context[b,:] = att[b] @ enco[b]: stationary att column, moving
        # enco rows.  Each row lands in psum [1,400]; rows are packed into
        # partition 0 of rows_sb, then one SBUF->SBUF DMA restores batch
        # layout.
        front_ctx = ExitStack()
        psC = front_ctx.enter_context(tc.tile_pool(name="psC", bufs=3, space="PSUM"))
        rowsp = front_ctx.enter_context(tc.tile_pool(name="rowsp", bufs=1))
        rows_sb = rowsp.tile([1, BC, A], F32)
        eop = front_ctx.enter_context(tc.tile_pool(name="eop", bufs=4))
        for g in range(8):
            eo_sb = eop.tile([128, 16, A], enc_dt, tag="eo")
            nc.sync.dma_start(out=eo_sb, in_=enco[g])
            for bi in range(4):
                b = g * 4 + bi
                ps_row = psC.tile([1, A], F32, tag="psc")
                if ENCO_FP8:
                    for hf in range(2):
                        nc.tensor.matmul(
                            out=ps_row,
                            lhsT=attT_sb[:, 2 * hf:2 * hf + 2, b:b + 1],
                            rhs=eo_sb[:, bi * 4 + 2 * hf:bi * 4 + 2 * hf + 2, :],
                            start=(hf == 0),
                            stop=(hf == 1),
                            perf_mode=DR,
                        )
                else:
                    for t in range(4):
                        nc.tensor.matmul(
                            out=ps_row,
                            lhsT=attT_sb[:, t, b:b + 1],
                            rhs=eo_sb[:, bi * 4 + t, :],
                            start=(t == 0),
                            stop=(t == 3),
                        )
                if b % 2 == 0:
                    nc.scalar.copy(out=rows_sb[:, b, :], in_=ps_row)
                else:
                    nc.vector.tensor_copy(out=rows_sb[:, b, :], in_=ps_row)
        # context batch-major [32, 400] (scalar HWDGE ring; sync ring is busy
        # with the enco/fc2w streams and is FIFO)
        ctx_sb = const.tile([BC, A], F32)
        nc.scalar.dma_start(out=ctx_sb, in_=rows_sb)
        front_ctx.close()
        # NOTE: when ENCO_FP8, ctx_sb carries a S_ATT*S_ENC scale; the host
        # pre-divides fc1a and pg2 by that factor so no descale op is needed.
        # context feature-major (bf16) for fc1, chunks (128,128,128,16)
        ctxT_bf = const.tile([128, 4, BC], BF16)
        for t, (e0, esz) in enumerate(CH4):
            ps_t = psT.tile([128, BC], F32, tag="tp")
            nc.tensor.transpose(ps_t[:esz, :], ctx_sb[:, e0:e0 + esz], ident[:BC, :BC])
            nc.scalar.copy(out=ctxT_bf[:esz, t, :], in_=ps_t[:esz, :])

        # ---- p_gen = sigmoid(x0.pg1 + ctx.pg2 + h.pg3) ----
        dot_tmp = small.tile([BC, H], F32, tag="dtmp", bufs=2)
        acc1 = small.tile([BC, 1], F32)
        nc.vector.scalar_tensor_tensor(
            out=dot_tmp[:, :I], in0=x0_sb, scalar=1.0, in1=pg1_sb,
            op0=ALU.mult, op1=ALU.mult, accum_out=acc1,
        )
        dot_tmp2 = small.tile([BC, H], F32, tag="dtmp", bufs=2)
        acc2 = small.tile([BC, 1], F32)
        nc.vector.scalar_tensor_tensor(
            out=dot_tmp2[:, :A], in0=ctx_sb, scalar=1.0, in1=pg2_sb,
            op0=ALU.mult, op1=ALU.mult, accum_out=acc2,
        )
        dot_tmp3 = small.tile([BC, H], F32, tag="dtmp", bufs=2)
        acc3 = small.tile([BC, 1], F32)
        nc.vector.scalar_tensor_tensor(
            out=dot_tmp3, in0=h_sb, scalar=1.0, in1=pg3_sb,
            op0=ALU.mult, op1=ALU.mult, accum_out=acc3,
        )
        nc.vector.tensor_add(out=acc1, in0=acc1, in1=acc2)
        nc.vector.tensor_add(out=acc1, in0=acc1, in1=acc3)
        # gen = sigmoid(acc) via exp (reuses the exp table loaded for softmax,
        # avoiding two 1.3us activation-table reloads on the critical path)
        gexp = small.tile([BC, 1], F32)
        nc.scalar.activation(out=gexp, in_=acc1, func=AF.Exp, scale=-1.0)
        gden = small.tile([BC, 1], F32)
        nc.vector.tensor_scalar_add(out=gden, in0=gexp, scalar1=1.0)
        gen_sb = small.tile([BC, 1], F32)
        nc.vector.reciprocal(out=gen_sb, in_=gden)  # 1/(1+e^-x)
        nc.scalar.dma_start(out=gen_out[:], in_=gen_sb)
        gen1m = small.tile([BC, 1], F32)
        nc.vector.tensor_mul(out=gen1m, in0=gexp, in1=gen_sb)  # e^-x/(1+e^-x)

        # att_copy = (1-gen) * att  -> output
        attcopy_sb = const.tile([BC, A], F32)
        nc.vector.tensor_scalar_mul(out=attcopy_sb, in0=att_sb[:, 0:A], scalar1=gen1m)
        nc.scalar.dma_start(out=attcopy_out[:], in_=attcopy_sb)

        # ---- fc1: z1 = [ctx | h] @ fc1_w^T (+ fc1_b) ----
        # When FC2_FP8 the z1 AllGather payload is fp8 (scale S_Z1 folded in
        # here), halving the collective's bytes.
        z1_dt = FP8 if FC2_FP8 else BF16
        z1_s = S_Z1 if FC2_FP8 else 1.0
        z1g_sb = const.tile([BC, 1024], z1_dt)
        for nh in range(2):
            ps_z = psA.tile([BC, NT], F32, tag="mmA")
            ns = slice(nh * NT, (nh + 1) * NT)
            for k, (c0, csz) in enumerate(CH4):
                nc.tensor.matmul(
                    out=ps_z, lhsT=ctxT_bf[:csz, k, :], rhs=fc1a_sb[:csz, k, ns],
                    start=(k == 0), stop=False,
                )
            for k in range(4):
                nc.tensor.matmul(
                    out=ps_z, lhsT=hT_bf[:, k, :], rhs=fc1b_sb[:, k, ns],
                    start=False, stop=(k == 3),
                )
            if with_fc1_bias:
                # host pre-scales fc1bias by z1_s
                nc.vector.scalar_tensor_tensor(
                    out=z1g_sb[:, ns], in0=ps_z, scalar=z1_s,
                    in1=fc1bias_sb[:, ns], op0=ALU.mult, op1=ALU.add,
                )
            elif nh == 0:
                nc.scalar.activation(out=z1g_sb[:, ns], in_=ps_z, func=AF.Identity, scale=z1_s)
            else:
                nc.vector.tensor_scalar_mul(out=z1g_sb[:, ns], in0=ps_z, scalar1=z1_s)
        nc.gpsimd.dma_start(out=z1g_c[:], in_=z1g_sb)

        # ---- AllGather z1 across the 8 cores (bf16, 64KB -> 512KB) ----
        nc.gpsimd.collective_compute(
            "AllGather", ALU.bypass, replica_groups=RG,
            ins=[z1g_c.opt()], outs=[z1g_full.opt()],
        )

        # z1^T [128, 8, 256] via on-chip transposes (dtype matches the AG)
        if FC2_FP8:
            ident_f8 = const.tile([128, 128], FP8)
            make_identity(nc, ident_f8)
            t_ident = ident_f8
        else:
            t_ident = ident_bf
        z1T_bh = []
        for bh in range(2):
            z1T_half = const.tile([128, 8, 128], ag_dt, tag=f"z1T{bh}")
            z1T_bh.append(z1T_half)
        zrp = ctx.enter_context(tc.tile_pool(name="zrp", bufs=2))
        for bh in range(2):
            zrow = zrp.tile([128, 1024], ag_dt, tag="zrow")
            nc.scalar.dma_start(out=zrow, in_=z1g_full[bh * 128:(bh + 1) * 128, :])
            for k2 in range(4):
                if FC2_FP8:
                    # fp8 transpose mode requires output element step of 2
                    ps_t = psT.tile([128, 2, 128, 2], ag_dt, tag="tpz")
                    ps_v = ps_t[:, :, :, 0]
                else:
                    ps_t = psT.tile([128, 2, 128], ag_dt, tag="tpz")
                    ps_v = ps_t
                for j in range(2):
                    nc.tensor.transpose(
                        ps_v[:, j, :], zrow[:, (k2 * 2 + j) * 128:(k2 * 2 + j + 1) * 128], t_ident
                    )
                if k2 % 2 == 0:
                    nc.scalar.copy(
                        out=z1T_bh[bh][:, k2 * 2:k2 * 2 + 2, :], in_=ps_v
                    )
                else:
                    nc.vector.tensor_copy(
                        out=z1T_bh[bh][:, k2 * 2:k2 * 2 + 2, :], in_=ps_v
                    )

        # ---- fc2: exp(logits) per vocab tile + partial denominators ----
        # Weight tiles stream on the sync ring (queued behind enco; nothing
        # else follows on that ring so head-of-line blocking is harmless).
        exp_scale = 1.0 / (S_Z1 * S_W2) if FC2_FP8 else 1.0
        s_acc = small.tile([128, 2], F32, tag="sacc")
        nc.vector.memset(s_acc, 0.0)
        wp_bufs = NVT if FC2_FP8 else 8
        wp = ctx.enter_context(tc.tile_pool(name="wp", bufs=wp_bufs))
        psD = ctx.enter_context(tc.tile_pool(name="psD", bufs=3, space="PSUM"))
        op_ = ctx.enter_context(tc.tile_pool(name="op", bufs=3))
        # All weight tiles are resident (bufs=NVT); the compute loop walks
        # tiles in groups of 3 with the k-pair loop outermost inside a group,
        # so each z1T stationary block is loaded once per 3 tiles instead of
        # per (tile, bh) -- LDWEIGHTS count drops ~3x.
        wn_tiles = []
        for t in range(NVT):
            wn_sb = wp.tile([128, 8, NT], w2_dt, tag=f"wn{t}", bufs=1)
            nc.sync.dma_start(out=wn_sb, in_=fc2w[t])
            wn_tiles.append(wn_sb)
        o_dt = FP8 if FC2_FP8 else BF16
        for tg0 in range(0, NVT, 3):
            tg = range(tg0, min(tg0 + 3, NVT))
            for bh in range(2):
                pss = {}
                pool = psA if ((tg0 // 3 + bh) % 2 == 0) else psD
                for t in tg:
                    ps_l = pool.tile([128, NT], F32, tag="mmA" if pool is psA else "mmD")
                    pss[t] = ps_l
                if FC2_FP8:
                    for kp in range(4):
                        for t in tg:
                            nc.tensor.matmul(
                                out=pss[t],
                                lhsT=z1T_bh[bh][:, 2 * kp:2 * kp + 2, :],
                                rhs=wn_tiles[t][:, 2 * kp:2 * kp + 2, :],
                                start=(kp == 0), stop=(kp == 3),
                                perf_mode=DR,
                            )
                else:
                    for k in range(8):
                        for t in tg:
                            nc.tensor.matmul(
                                out=pss[t],
                                lhsT=z1T_bh[bh][:, k, :],
                                rhs=wn_tiles[t][:, k, :],
                                start=(k == 0), stop=(k == 7),
                            )
                for t in tg:
                    o_sb = op_.tile([128, NT], o_dt, tag="osb")
                    st = small.tile([128, 1], F32, tag="st")
                    nc.scalar.activation(
                        out=o_sb, in_=pss[t], func=AF.Exp, scale=exp_scale, accum_out=st,
                    )
                    nc.vector.tensor_add(
                        out=s_acc[:, bh:bh + 1], in0=s_acc[:, bh:bh + 1], in1=st
                    )
                    nc.scalar.dma_start(out=p_img[bh, :, t, :], in_=o_sb)
        for bh in range(2):
            nc.scalar.dma_start(
                out=s_out[bh * 128:(bh + 1) * 128, :], in_=s_acc[:, bh:bh + 1]
            )

    nc.compile()
    return nc


_NC_CACHE = {}


def _get_nc(with_fc1_bias: bool) -> bass.Bass:
    if with_fc1_bias not in _NC_CACHE:
        _NC_CACHE[with_fc1_bias] = build_nc(with_fc1_bias)
    return _NC_CACHE[with_fc1_bias]


RUN_KW = {}        # test.py can set e.g. {"trace": True}
LAST_RESULT = {}   # test.py reads exec_time_ns etc.


def _fp8(a):
    return np.clip(a, -240.0, 240.0).astype(ml_dtypes.float8_e4m3)


def make_in_maps(inputs: dict):
    f32 = lambda a: np.ascontiguousarray(np.asarray(a), dtype=np.float32)
    bf16 = ml_dtypes.bfloat16

    x = f32(inputs["x"])[:, 0, :]              # [B, I]
    enco = f32(inputs["encoder_outputs"])      # [B, A, A]
    es = f32(inputs["encoder_state"])          # [B, H]
    W_ih = f32(inputs["W_ih"])                 # [4H, I]
    b = f32(inputs["b_ih"]) + f32(inputs["b_hh"])  # [4H]
    Wh_w = f32(inputs["Wh_w"])                 # [A, H]
    Ws_w = f32(inputs["Ws_w"])
    attb = f32(inputs["Wh_b"]) + f32(inputs["Ws_b"])  # [A]
    vvec = f32(inputs["v"])                    # [A]
    fc1_w = f32(inputs["fc1_w"])               # [2H, H+A]
    fc1_b = f32(inputs["fc1_b"])               # [2H]
    fc2_w = f32(inputs["fc2_w"])               # [V, 2H]
    pg1 = f32(inputs["pg1_w"])[0]              # [I]
    pg2 = f32(inputs["pg2_w"])[0]              # [A]
    pg3 = f32(inputs["pg3_w"])[0]              # [H]

    with_fc1_bias = bool(np.any(fc1_b != 0.0))

    # i, g, o gate rows of W_ih / bias (f gate is dead: c0 = 0)
    idx = np.r_[0:H, 2 * H:3 * H, 3 * H:4 * H]
    wihT = np.ascontiguousarray(W_ih[idx].T).astype(bf16)   # [I, 1536]
    bg = np.ascontiguousarray(b[idx])          # [1536]
    bgT = bg.reshape(12, 128).T.astype(bf16)   # [128, 12]

    whsw = np.concatenate([Wh_w.T, Ws_w.T], axis=1).astype(bf16)  # [H, 2A]
    # shared part of the packed small-weight image
    swb = np.zeros((128, NW), dtype=bf16)
    swb[:, SW_WIH:SW_WIH + 1536] = wihT[0:128]
    swb[:, SW_WIH + 1536:SW_WIH + 3072] = wihT[128:256]
    swb[:, SW_BG:SW_BG + 12] = bgT
    for k in range(4):
        swb[:, SW_WHSW + k * 800:SW_WHSW + (k + 1) * 800] = whsw[k * 128:(k + 1) * 128]
    # when ENCO_FP8 the on-chip context carries a S_ATT*S_ENC scale; fold the
    # descale into its consumers (fc1's ctx rows and the pg2 gate weights)
    ctx_ds = 1.0 / (S_ATT * S_ENC) if ENCO_FP8 else 1.0
    smallp = np.concatenate([attb, vvec, pg2 * ctx_ds, pg1, pg3])  # [1968]
    fc1T = fc1_w.T                              # [H+A, 2H]
    # fc1 images: a-chunks (128,128,128,16 padded), h-chunks (4x128)
    fc1ab_img = np.zeros((128, 8, 2 * H), dtype=bf16)
    for t, (e0, esz) in enumerate(CH4):
        fc1ab_img[:esz, t, :] = (fc1T[e0:e0 + esz] * ctx_ds).astype(bf16)
    fc1ab_img[:, 4:8, :] = fc1T[A:].reshape(4, 128, 2 * H).transpose(1, 0, 2).astype(bf16)

    # fc2 weight image: [core][tile][p][kchunk][j], vocab padded 6250->6656
    if FC2_FP8:
        fc2c = _fp8(fc2_w * S_W2)
        fc2_pad = np.zeros((NCORES * VCP, 2 * H), dtype=ml_dtypes.float8_e4m3)
    else:
        fc2c = fc2_w.astype(bf16)
        fc2_pad = np.zeros((NCORES * VCP, 2 * H), dtype=bf16)
    fc2_pad.reshape(NCORES, VCP, 2 * H)[:, :VC, :] = fc2c.reshape(NCORES, VC, 2 * H)
    fc2_img = np.ascontiguousarray(
        fc2_pad.reshape(NCORES, NVT, NT, 8, 128).transpose(0, 1, 4, 3, 2)
    )  # [8, NVT, 128, 8, NT]

    # enco image: [64 groups][p][b4*4 + achunk][e], a padded 400->512
    if ENCO_FP8:
        enco_c = _fp8(enco * S_ENC)
        enco_pad = np.zeros((B, 512, A), dtype=ml_dtypes.float8_e4m3)
    else:
        enco_c = enco.astype(bf16)
        enco_pad = np.zeros((B, 512, A), dtype=bf16)
    enco_pad[:, :A, :] = enco_c
    enco_img = np.ascontiguousarray(
        enco_pad.reshape(64, 4, 4, 128, A).transpose(0, 3, 1, 2, 4)
    )  # [64, 128, 4(b), 4(chunk), A]
    enco_img = enco_img.reshape(64, 128, 16, A)

    x0T = np.ascontiguousarray(x.T)             # [I, B]
    esT = np.ascontiguousarray(es.T)            # [H, B]

    in_maps = []
    for c in range(NCORES):
        bs = slice(c * BC, (c + 1) * BC)
        sw = swb.copy()
        x0Tc = x0T[:, bs].astype(bf16)
        esTc = esT[:, bs].astype(bf16)
        sw[:, SW_X0T:SW_X0T + BC] = x0Tc[0:128]
        sw[:, SW_X0T + BC:SW_X0T + 2 * BC] = x0Tc[128:256]
        for k in range(4):
            sw[:, SW_EST + k * BC:SW_EST + (k + 1) * BC] = esTc[k * 128:(k + 1) * 128]
        m = {
            "x0": np.ascontiguousarray(x[bs]),
            "smallw": sw,
            "enco": enco_img[c * 8:(c + 1) * 8],
            "smallp": smallp,
            "fc1ab": fc1ab_img,
            "fc2w": fc2_img[c],
        }
        if with_fc1_bias:
            m["fc1bias"] = fc1_b * (S_Z1 if FC2_FP8 else 1.0)
        in_maps.append(m)
    return in_maps, with_fc1_bias


def kernel(**inputs) -> np.ndarray:
    in_maps, with_fc1_bias = make_in_maps(inputs)
    nc = _get_nc(with_fc1_bias)

    res = run_bass_kernel_spmd(nc, in_maps, core_ids=list(range(NCORES)), **RUN_KW)
    results = res.results
    LAST_RESULT["exec_time_ns"] = getattr(res, "exec_time_ns", None)
    LAST_RESULT["mean_exec_time_ns"] = getattr(res, "mean_exec_time_ns", None)
    LAST_RESULT["max_exec_time_core_id"] = getattr(res, "max_exec_time_core_id", None)

    oov = int(np.asarray(inputs["max_oov_nums"]))
    ids = np.asarray(inputs["ids"])
    fc2_b = np.asarray(inputs["fc2_b"], dtype=np.float32)
    npad = VCP - VC  # zero-weight pad columns per core: exp(0)=1 each

    gen = np.concatenate([np.asarray(results[c]["gen_out"])[:, 0] for c in range(NCORES)])
    pimgs = [
        np.asarray(results[c]["p_img"], dtype=np.float32).reshape(B, VCP)
        for c in range(NCORES)
    ]
    p = np.zeros((B, V + oov), dtype=np.float32)
    if np.any(fc2_b != 0.0):
        # device computed exp(z); fold exp(fc2_b) in and recompute denominators
        g = np.exp(fc2_b.astype(np.float64)).astype(np.float32)
        for c in range(NCORES):
            vs = slice(c * VC, (c + 1) * VC)
            p[:, vs] = pimgs[c][:, :VC] * g[None, vs]
        s = p[:, :V].sum(axis=1)
        p[:, :V] *= (gen / s)[:, None]
    else:
        s = np.zeros(B, dtype=np.float32)
        for c in range(NCORES):
            s += np.asarray(results[c]["s_out"])[:, 0]
        s -= NCORES * npad  # remove the pad columns' exp(0) contributions
        f = (gen / s).astype(np.float32)
        for c in range(NCORES):
            vs = slice(c * VC, (c + 1) * VC)
            p[:, vs] = pimgs[c][:, :VC] * f[:, None]

    att_copy = np.concatenate(
        [np.asarray(results[c]["attcopy_out"]) for c in range(NCORES)], axis=0
    )
    rows = np.arange(B)[:, None]
    np.add.at(p, (rows, ids), att_copy)
    return p
